# revision 22
# baseline (speedup 1.0000x reference)
"""Trainium2 Bass kernel for nn_MixtureOfBidders.

Data-parallel over tokens (8 cores x 512 tokens), weights replicated.
On-device layout is transposed: [feature partitions, token free-dim].

v3 design (vs v1 baseline at 1.63 ms):
  - all heavy matmuls in bf16 (fp32r is ~1.5x slower on the PE and blocks
    fast weight load); conf/auction stays fp32 to match reference top-k
  - weights pre-packed on host chunk-major so every weight DMA is one
    fully-contiguous block per partition with a small (<=32 KiB)
    partition stride (large partition strides corrupt DMA on HW)
  - routing (top-2 auction) via PE transposes + free-dim reductions in
    token-major layout -- no DRAM round-trips
  - per-expert combine weights broadcast to 128 partitions with one-hot
    matmuls instead of DRAM-bounce DMAs
  - down-LoRA rank partials accumulate in PSUM across all I-chunks
    (2 experts col-packed per bank; opened once by a zeroing matmul)
  - g = base + lora computed via identity-matmul accumulation in PSUM
    (PE-side add); u-path add runs ACT copy + DVE bf16 add
"""

import functools
import os
import sys

import numpy as np

sys.path.insert(0, "/opt/trn_rl_repo")

import ml_dtypes  # noqa: E402

import concourse.bass as bass  # noqa: E402
from concourse import bacc  # noqa: E402
import concourse.mybir as mybir  # noqa: E402
import concourse.tile as tile  # noqa: E402
from concourse.bass_utils import run_bass_kernel_spmd  # noqa: E402

B, S, H, I, E, TOPK, R = 4, 1024, 2048, 7168, 8, 2, 64
SCALING = 16.0 / 64.0
N_CORES = 8
N_TOK = B * S  # 4096
T = N_TOK // N_CORES  # 512 tokens per core
TC = T // 128  # 4 token chunks for transposed routing
HC = H // 128  # 16 contraction chunks over H
IT = I // 128  # 56 chunks over I
IT2 = IT // 2
HC2 = HC // 2
E2 = E // 2
TDPSUM = os.environ.get("TDPSUM", "1") == "1"

F32 = mybir.dt.float32
F32R = mybir.dt.float32r
BF16 = mybir.dt.bfloat16
BFNP = ml_dtypes.bfloat16
AF = mybir.ActivationFunctionType
OP = mybir.AluOpType
AX = mybir.AxisListType


def build_module() -> bass.Bass:
    nc = bacc.Bacc("TRN2", target_bir_lowering=False)

    # ---- dram I/O (per core) ----
    xT32 = nc.dram_tensor("xT32", [H, T], F32R, kind="ExternalInput")
    xTb = nc.dram_tensor("xTb", [H, T], BF16, kind="ExternalInput")
    conf_wt = nc.dram_tensor("conf_wt", [H, E], F32, kind="ExternalInput")
    conf_b = nc.dram_tensor("conf_b", [E, 1], F32, kind="ExternalInput")
    wealth = nc.dram_tensor("wealth", [E, 1], F32, kind="ExternalInput")
    # packed weights (see _host_prep for layouts); leading dim is the
    # chunk index so each load has a small partition stride
    guA = nc.dram_tensor("guA", [128, HC, E, 128], BF16, kind="ExternalInput")
    guBg = nc.dram_tensor("guBg", [IT2, 64, 2, E, 128], BF16, kind="ExternalInput")
    guBu = nc.dram_tensor("guBu", [IT2, 64, 2, E, 128], BF16, kind="ExternalInput")
    bgp = nc.dram_tensor("bgp", [IT2, 128, 2, HC, 128], BF16, kind="ExternalInput")
    bup = nc.dram_tensor("bup", [IT2, 128, 2, HC, 128], BF16, kind="ExternalInput")
    bdp = nc.dram_tensor("bdp", [HC2, 128, 2, IT, 128], BF16, kind="ExternalInput")
    dAp = nc.dram_tensor("dAp", [IT2, 128, 2, E, R], BF16, kind="ExternalInput")
    dBE = E2 if TDPSUM else E
    dBK = 128 if TDPSUM else 64
    dBp = nc.dram_tensor("dBp", [HC2, dBK, 2, dBE, 128], BF16, kind="ExternalInput")
    ident = nc.dram_tensor("ident", [128, 128], BF16, kind="ExternalInput")
    idf = nc.dram_tensor("idf", [128, 128], F32, kind="ExternalInput")
    bcast8 = nc.dram_tensor("bcast8", [E, E, 128], BF16, kind="ExternalInput")
    outT = nc.dram_tensor("outT", [H, T], F32, kind="ExternalOutput")

    with tile.TileContext(nc) as tc:
        with (
            tc.tile_pool(name="consts", bufs=1) as consts,
            tc.tile_pool(name="acc", bufs=IT) as accp,
            tc.tile_pool(name="xp", bufs=1) as xp,
            tc.tile_pool(name="tA", bufs=E) as tAp,
            tc.tile_pool(name="web", bufs=E) as webp,
            tc.tile_pool(name="tds", bufs=E) as tdsp,
        ):
            id_sb = consts.tile([128, 128], BF16)
            nc.sync.dma_start(out=id_sb, in_=ident[:, :])
            idf_sb = consts.tile([128, 128], F32)
            nc.sync.dma_start(out=idf_sb, in_=idf[:, :])
            bc8_sb = consts.tile([E, E, 128], BF16)
            nc.sync.dma_start(out=bc8_sb, in_=bcast8[:, :, :])
            cb_sb = consts.tile([E, 1], F32)
            nc.sync.dma_start(out=cb_sb, in_=conf_b[:, :])
            wl_sb = consts.tile([E, 1], F32)
            nc.sync.dma_start(out=wl_sb, in_=wealth[:, :])

            acc_t = [
                accp.tile([128, T], BF16, tag="acc", name=f"acc{i}")
                for i in range(IT)
            ]

            # ---------- load x (bf16 for compute) ----------
            xb_sb = xp.tile([128, HC, T], BF16)
            nc.sync.dma_start(
                out=xb_sb, in_=xTb[:, :].rearrange("(c p) t -> p c t", p=128)
            )

            we_b = []
            tAg = []
            tAu = []
            with (
                tc.tile_pool(name="rt", bufs=2) as rt,
                tc.tile_pool(name="xf", bufs=2) as xf,
                tc.tile_pool(name="wga", bufs=1) as wga,
                tc.tile_pool(name="prt", bufs=4, space="PSUM") as prt,
                tc.tile_pool(name="prs", bufs=2, space="PSUM") as prs,
            ):
                # ---------- tA = x @ [gate_A | up_A]: split 64-row tiles ----
                ga_sb = wga.tile([128, HC, E, 128], BF16, tag="guA")
                nc.sync.dma_start(out=ga_sb, in_=guA[:, :, :, :])
                for e in range(E):
                    p_tA = prt.tile([128, T], F32, tag="pbig")
                    for hc in range(HC):
                        nc.tensor.matmul(
                            p_tA,
                            ga_sb[:, hc, e, :],
                            xb_sb[:, hc, :],
                            start=(hc == 0),
                            stop=(hc == HC - 1),
                        )
                    tg = tAp.tile([64, T], BF16, tag="tAg", name=f"tAg{e}")
                    nc.scalar.copy(tg, p_tA[0:64, :])
                    tAg.append(tg)
                    tu = tAp.tile([64, T], BF16, tag="tAu", name=f"tAu{e}")
                    nc.scalar.copy(tu, p_tA[64:128, :])
                    tAu.append(tu)

                # ---------- confidence head (fp32, matches reference) ------
                cw_sb = rt.tile([128, HC, E], F32, tag="cw")
                nc.sync.dma_start(
                    out=cw_sb,
                    in_=conf_wt[:, :].rearrange("(c p) e -> p c e", p=128),
                )
                p_cf = prt.tile([128, T], F32, tag="pbig")
                for hc in range(HC):
                    xf_c = xf.tile([128, T], F32R, tag="xf")
                    nc.sync.dma_start(
                        out=xf_c, in_=xT32[hc * 128 : (hc + 1) * 128, :]
                    )
                    nc.tensor.matmul(
                        p_cf[0:E, :],
                        cw_sb[:, hc, :],
                        xf_c.bitcast(F32),
                        start=(hc == 0),
                        stop=(hc == HC - 1),
                    )
                conf = rt.tile([E, T], F32, tag="conf")
                nc.scalar.activation(conf, p_cf[0:E, :], AF.Sigmoid, bias=cb_sb)
                bids = rt.tile([E, T], F32, tag="bids")
                nc.vector.tensor_scalar(bids, conf, wl_sb, None, op0=OP.mult)

                # ---------- top-2 auction in token-major layout ----------
                # transpose bids into [128 tokens, E] chunks, reduce over the
                # free dim, build masks with per-partition scalars, transpose
                # the combine weights back.  No DRAM round trips.
                bT = rt.tile([128, TC, E], F32, tag="bT")
                m1 = rt.tile([128, TC], F32, tag="m1")
                m2 = rt.tile([128, TC], F32, tag="m2")
                msk1 = rt.tile([128, TC, E], F32, tag="msk1")
                msk2 = rt.tile([128, TC, E], F32, tag="msk2")
                b2 = rt.tile([128, TC, E], F32, tag="b2")
                weT = rt.tile([128, TC, E], F32, tag="weT")
                we8 = rt.tile([E, T], BF16, tag="we8")
                for c in range(TC):
                    p_bT = prs.tile([128, E], F32, tag="ptr")
                    nc.tensor.transpose(
                        p_bT, bids[:, c * 128 : (c + 1) * 128], idf_sb[0:E, 0:E]
                    )
                    nc.vector.tensor_copy(bT[:, c, :], p_bT)
                    nc.vector.tensor_reduce(
                        m1[:, c : c + 1], bT[:, c, :], AX.X, OP.max
                    )
                    nc.vector.tensor_scalar(
                        msk1[:, c, :], bT[:, c, :], m1[:, c : c + 1], None,
                        op0=OP.is_equal,
                    )
                    nc.vector.scalar_tensor_tensor(
                        b2[:, c, :], msk1[:, c, :], -1e6, bT[:, c, :],
                        op0=OP.mult, op1=OP.add,
                    )
                    nc.vector.tensor_reduce(
                        m2[:, c : c + 1], b2[:, c, :], AX.X, OP.max
                    )
                    nc.vector.tensor_scalar(
                        msk2[:, c, :], b2[:, c, :], m2[:, c : c + 1], None,
                        op0=OP.is_equal,
                    )
                d12 = rt.tile([128, TC], F32, tag="d12")
                nc.vector.tensor_sub(d12, m1, m2)
                w1 = rt.tile([128, TC], F32, tag="w1")
                nc.scalar.activation(w1, d12, AF.Sigmoid)
                w2 = rt.tile([128, TC], F32, tag="w2")
                nc.scalar.activation(w2, d12, AF.Sigmoid, scale=-1.0)
                for c in range(TC):
                    t1 = rt.tile([128, E], F32, tag="t1")
                    nc.vector.tensor_scalar(
                        t1, msk1[:, c, :], w1[:, c : c + 1], None, op0=OP.mult
                    )
                    t2 = rt.tile([128, E], F32, tag="t2")
                    nc.vector.tensor_scalar(
                        t2, msk2[:, c, :], w2[:, c : c + 1], None, op0=OP.mult
                    )
                    nc.vector.tensor_add(weT[:, c, :], t1, t2)
                    p_weT = prs.tile([E, 128], F32, tag="ptrb")
                    nc.tensor.transpose(p_weT, weT[:, c, :], idf_sb)
                    nc.vector.tensor_copy(
                        we8[:, c * 128 : (c + 1) * 128], p_weT
                    )
                # broadcast each expert's weight row to 128 partitions via
                # one-hot matmuls
                for e in range(E):
                    p_web = prt.tile([128, T], F32, tag="pbig")
                    nc.tensor.matmul(
                        p_web, bc8_sb[:, e, :], we8, start=True, stop=True
                    )
                    wt = webp.tile([128, T], BF16, tag="web", name=f"web{e}")
                    nc.scalar.copy(wt, p_web)
                    we_b.append(wt)

            # ---------- main loop over I chunk-pairs ----------
            with (
                tc.tile_pool(name="wgw", bufs=2) as wgw,
                tc.tile_pool(name="wb", bufs=2) as wbp,
                tc.tile_pool(name="wdA", bufs=3) as wdAp,
                tc.tile_pool(name="bsb", bufs=4) as bsb,
                tc.tile_pool(name="ew", bufs=4) as ew,
                tc.tile_pool(name="hwp", bufs=E + 1) as hwp,
                tc.tile_pool(name="ptd", bufs=E2, space="PSUM") as ptd,
                tc.tile_pool(name="pw", bufs=4, space="PSUM") as pw,
            ):
                if TDPSUM:
                    # open each down-LoRA PSUM bank once with a zeroing
                    # matmul (sets has_written across all 128 partitions);
                    # the per-expert dA matmuls then accumulate with
                    # start=False in their own partition halves.
                    zro = bsb.tile([128, 128], BF16, tag="zro")
                    nc.vector.memset(zro, 0)
                    p_td = [
                        ptd.tile([128, T], F32, tag="ptd", name=f"ptd{p}")
                        for p in range(E2)
                    ]
                    for p in range(E2):
                        nc.tensor.matmul(
                            p_td[p],
                            zro,
                            xb_sb[:, 0, :],
                            start=True,
                            stop=False,
                            skip_group_check=True,
                        )
                else:
                    td_bf = [
                        tdsp.tile([64, T], BF16, tag="tds", name=f"tds{q}")
                        for q in range(E)
                    ]
                for it2 in range(IT2):
                    bg_w = wgw.tile([128, 2, HC, 128], BF16, tag="bgw")
                    nc.sync.dma_start(out=bg_w, in_=bgp[it2, :, :, :, :])
                    bu_w = wgw.tile([128, 2, HC, 128], BF16, tag="buw")
                    nc.sync.dma_start(out=bu_w, in_=bup[it2, :, :, :, :])
                    gBg_s = wbp.tile([64, 2, E, 128], BF16, tag="gBg")
                    nc.sync.dma_start(out=gBg_s, in_=guBg[it2, :, :, :, :])
                    gBu_s = wbp.tile([64, 2, E, 128], BF16, tag="gBu")
                    nc.sync.dma_start(out=gBu_s, in_=guBu[it2, :, :, :, :])
                    dA_s = wdAp.tile([128, 2, E, R], BF16, tag="dA")
                    nc.sync.dma_start(out=dA_s, in_=dAp[it2, :, :, :, :])

                    for j in range(2):
                        it = 2 * it2 + j
                        hw_list = []
                        p_bg = pw.tile([128, T], F32, tag="big")
                        p_bu = pw.tile([128, T], F32, tag="big")
                        for hc in range(HC):
                            nc.tensor.matmul(
                                p_bg,
                                bg_w[:, j, hc, :],
                                xb_sb[:, hc, :],
                                start=(hc == 0),
                                stop=(hc == HC - 1),
                            )
                        for hc in range(HC):
                            nc.tensor.matmul(
                                p_bu,
                                bu_w[:, j, hc, :],
                                xb_sb[:, hc, :],
                                start=(hc == 0),
                                stop=(hc == HC - 1),
                            )
                        bg_s = bsb.tile([128, T], BF16, tag="bgs")
                        nc.scalar.copy(bg_s, p_bg)
                        bu_s = bsb.tile([128, T], BF16, tag="bus")
                        nc.scalar.copy(bu_s, p_bu)

                        for e in range(E):
                            # g = base_g + lora_g in PSUM (identity trick;
                            # keeps the PSUM bank consumer on ACT so banks
                            # recycle fast -- a DVE-side add stalls the PE)
                            p_g = pw.tile([128, T], F32, tag="big")
                            nc.tensor.matmul(
                                p_g, id_sb, bg_s, start=True, stop=False
                            )
                            nc.tensor.matmul(
                                p_g,
                                gBg_s[:, j, e, :],
                                tAg[e],
                                start=False,
                                stop=True,
                            )
                            p_lu = pw.tile([128, T], F32, tag="big")
                            nc.tensor.matmul(
                                p_lu,
                                gBu_s[:, j, e, :],
                                tAu[e],
                                start=True,
                                stop=True,
                            )
                            sg = ew.tile([128, T], BF16, tag="sg")
                            nc.scalar.activation(sg, p_g, AF.Silu)
                            # u = lora_u + base_u; alternate the engine per
                            # expert parity so neither ACT nor DVE lags the
                            # PE inside the 8-expert stretch
                            u_t = ew.tile([128, T], BF16, tag="u")
                            if e % 2 == 0:
                                lu_s = ew.tile([128, T], BF16, tag="lu")
                                nc.scalar.copy(lu_s, p_lu)
                                nc.vector.tensor_add(u_t, lu_s, bu_s)
                            else:
                                nc.vector.scalar_tensor_tensor(
                                    u_t, p_lu, 1.0, bu_s,
                                    op0=OP.bypass, op1=OP.add,
                                )
                            h_t = ew.tile([128, T], BF16, tag="h")
                            nc.vector.tensor_mul(h_t, sg, u_t)
                            hw_t = hwp.tile([128, T], BF16, tag="hw")
                            nc.vector.tensor_mul(hw_t, h_t, we_b[e])
                            if e == 0:
                                # snapshot copy: the batched dA matmul below
                                # must read expert 0's hw, not the running acc
                                nc.vector.tensor_copy(acc_t[it], hw_t)
                            else:
                                nc.vector.tensor_add(acc_t[it], acc_t[it], hw_t)
                            hw_list.append(hw_t)
                            if not TDPSUM:
                                p_t1 = pw.tile([64, T], F32, tag="ptd1")
                                nc.tensor.matmul(
                                    p_t1, dA_s[:, j, e, :], hw_t,
                                    start=True, stop=True,
                                )
                                if it == 0:
                                    nc.vector.tensor_copy(td_bf[e], p_t1)
                                else:
                                    nc.vector.tensor_add(
                                        td_bf[e], td_bf[e], p_t1
                                    )
                        if TDPSUM:
                            # batched dA matmuls: rhs tiles all ready, and
                            # same-bank runs minimize PSUM bank switches
                            for e in range(E):
                                nc.tensor.matmul(
                                    p_td[e // 2][
                                        (e % 2) * 64 : (e % 2) * 64 + 64, :
                                    ],
                                    dA_s[:, j, e, :],
                                    hw_list[e],
                                    start=False,
                                    stop=(it == IT - 1 and e == E - 1),
                                    skip_group_check=True,
                                )
                        hw_list.clear()

                # td: PSUM -> [128, T] bf16 expert-pair tiles (the dB
                # matmuls consume stacked pairs as K=128)
                if TDPSUM:
                    td_sb = []
                    for q in range(E2):
                        ts = tdsp.tile([128, T], BF16, tag="tds", name=f"tds{q}")
                        nc.scalar.copy(ts, p_td[q])
                        td_sb.append(ts)
                else:
                    td_sb = td_bf

            # ---------- down projection ----------
            with (
                tc.tile_pool(name="wd", bufs=2) as wd,
                tc.tile_pool(name="wdB", bufs=2) as wdB,
                tc.tile_pool(name="osb", bufs=3) as osb,
                tc.tile_pool(name="po", bufs=2, space="PSUM") as pop,
            ):
                for hc2 in range(HC2):
                    bd_s = wd.tile([128, 2, IT, 128], BF16, tag="bd")
                    nc.sync.dma_start(out=bd_s, in_=bdp[hc2, :, :, :, :])
                    dB_s = wdB.tile([dBK, 2, dBE, 128], BF16, tag="dB")
                    nc.sync.dma_start(out=dB_s, in_=dBp[hc2, :, :, :, :])
                    for j in range(2):
                        hc = 2 * hc2 + j
                        p_o = pop.tile([128, T], F32, tag="po")
                        for it in range(IT):
                            nc.tensor.matmul(
                                p_o,
                                bd_s[:, j, it, :],
                                acc_t[it],
                                start=(it == 0),
                                stop=False,
                            )
                        if TDPSUM:
                            for q in range(E2):
                                nc.tensor.matmul(
                                    p_o,
                                    dB_s[:, j, q, :],
                                    td_sb[q],
                                    start=False,
                                    stop=(q == E2 - 1),
                                )
                        else:
                            for e in range(E):
                                nc.tensor.matmul(
                                    p_o,
                                    dB_s[0:64, j, e, :],
                                    td_sb[e],
                                    start=False,
                                    stop=(e == E - 1),
                                )
                        o_s = osb.tile([128, T], F32, tag="o")
                        nc.scalar.copy(o_s, p_o)
                        nc.sync.dma_start(
                            out=outT[hc * 128 : (hc + 1) * 128, :], in_=o_s
                        )
    nc.compile()
    return nc


@functools.lru_cache(maxsize=1)
def _get_module():
    return build_module()


def _host_prep(inputs):
    f32 = np.float32
    x = np.ascontiguousarray(np.asarray(inputs["hidden_states"], f32)).reshape(
        N_TOK, H
    )
    gate_A = np.asarray(inputs["gate_A"], f32)
    gate_B = np.asarray(inputs["gate_B"], f32)
    up_A = np.asarray(inputs["up_A"], f32)
    up_B = np.asarray(inputs["up_B"], f32)
    down_A = np.asarray(inputs["down_A"], f32)
    down_B = np.asarray(inputs["down_B"], f32)
    bgate = np.asarray(inputs["base_gate"], f32)
    bup_w = np.asarray(inputs["base_up"], f32)
    bdown = np.asarray(inputs["base_down"], f32)

    # guA: [128, HC, E, 128]; [p, hc, e, r] = concat(A)[e, hc*128+p, r]
    guA_c = np.concatenate([gate_A, up_A], axis=2)  # [E, H, 2R]
    guAp = np.ascontiguousarray(
        guA_c.reshape(E, HC, 128, 2 * R).transpose(2, 1, 0, 3).astype(BFNP)
    )

    # guBg/guBu: [IT2, 64, 2, E, 128] (scaled)
    def pack_guB(w):  # [E, R, I]
        w = w * f32(SCALING)
        return np.ascontiguousarray(
            w.reshape(E, R, IT2, 2, 128).transpose(2, 1, 3, 0, 4).astype(BFNP)
        )

    guBgp = pack_guB(gate_B)
    guBup = pack_guB(up_B)

    # base gate/up: [IT2, 128, 2, HC, 128];
    # [it2, p, j, hc, i] = W[hc*128+p, (2*it2+j)*128+i]
    def pack_base(w):  # [H, I]
        return np.ascontiguousarray(
            w.reshape(HC, 128, IT2, 2, 128).transpose(2, 1, 3, 0, 4).astype(BFNP)
        )

    bgpk = pack_base(bgate)
    bupk = pack_base(bup_w)
    # base down: [HC2, 128, 2, IT, 128];
    # [hc2, p, j, it, h] = W[it*128+p, (2*hc2+j)*128+h]
    bdpk = np.ascontiguousarray(
        bdown.reshape(IT, 128, HC2, 2, 128).transpose(2, 1, 3, 0, 4).astype(BFNP)
    )
    # down_A: [IT2, 128, 2, E, R]
    dApk = np.ascontiguousarray(
        down_A.reshape(E, IT2, 2, 128, R).transpose(1, 3, 2, 0, 4).astype(BFNP)
    )
    if TDPSUM:
        # down_B: [HC2, 128, 2, E2, 128]; rows 0-63 expert 2q, 64-127
        # expert 2q+1 (stacked pairs contract as K=128)
        dBpk = np.ascontiguousarray(
            (down_B * f32(SCALING))
            .reshape(E2, 2, R, HC2, 2, 128)
            .transpose(3, 1, 2, 4, 0, 5)
            .reshape(HC2, 128, 2, E2, 128)
            .astype(BFNP)
        )
    else:
        # down_B: [HC2, 64, 2, E, 128] (scaled)
        dBpk = np.ascontiguousarray(
            (down_B * f32(SCALING))
            .reshape(E, R, HC2, 2, 128)
            .transpose(2, 1, 3, 0, 4)
            .astype(BFNP)
        )
    bc8 = np.zeros((E, E, 128), dtype=BFNP)
    for e in range(E):
        bc8[e, e, :] = BFNP(1.0)

    shared = {
        "conf_wt": np.ascontiguousarray(np.asarray(inputs["conf_W"], f32).T),
        "conf_b": np.ascontiguousarray(
            np.asarray(inputs["conf_b"], f32).reshape(E, 1)
        ),
        "wealth": np.ascontiguousarray(
            np.asarray(inputs["expert_wealth"], f32).reshape(E, 1)
        ),
        "guA": guAp,
        "guBg": guBgp,
        "guBu": guBup,
        "bgp": bgpk,
        "bup": bupk,
        "bdp": bdpk,
        "dAp": dApk,
        "dBp": dBpk,
        "ident": np.eye(128, dtype=BFNP),
        "idf": np.eye(128, dtype=f32),
        "bcast8": bc8,
    }
    in_maps = []
    for c in range(N_CORES):
        m = dict(shared)
        xc = np.ascontiguousarray(x[c * T : (c + 1) * T, :].T)
        m["xT32"] = xc
        m["xTb"] = np.ascontiguousarray(xc.astype(BFNP))
        in_maps.append(m)
    return in_maps


def kernel(**inputs) -> np.ndarray:
    nc = _get_module()
    in_maps = _host_prep(inputs)
    res = run_bass_kernel_spmd(nc, in_maps, core_ids=list(range(N_CORES)))
    parts = [np.asarray(r["outT"], np.float32).T for r in res.results]
    return np.concatenate(parts, axis=0).reshape(B, S, H)


# revision 23
# speedup vs baseline: 1.0095x; 1.0095x over previous
"""Trainium2 Bass kernel for nn_MixtureOfBidders.

Data-parallel over tokens (8 cores x 512 tokens), weights replicated.
On-device layout is transposed: [feature partitions, token free-dim].

v3 design (vs v1 baseline at 1.63 ms):
  - all heavy matmuls in bf16 (fp32r is ~1.5x slower on the PE and blocks
    fast weight load); conf/auction stays fp32 to match reference top-k
  - weights pre-packed on host chunk-major so every weight DMA is one
    fully-contiguous block per partition with a small (<=32 KiB)
    partition stride (large partition strides corrupt DMA on HW)
  - routing (top-2 auction) via PE transposes + free-dim reductions in
    token-major layout -- no DRAM round-trips
  - per-expert combine weights broadcast to 128 partitions with one-hot
    matmuls instead of DRAM-bounce DMAs
  - down-LoRA rank partials accumulate in PSUM across all I-chunks
    (2 experts col-packed per bank; opened once by a zeroing matmul)
  - g = base + lora computed via identity-matmul accumulation in PSUM
    (PE-side add); u-path add runs ACT copy + DVE bf16 add
"""

import functools
import os
import sys

import numpy as np

sys.path.insert(0, "/opt/trn_rl_repo")

import ml_dtypes  # noqa: E402

import concourse.bass as bass  # noqa: E402
from concourse import bacc  # noqa: E402
import concourse.mybir as mybir  # noqa: E402
import concourse.tile as tile  # noqa: E402
from concourse.bass_utils import run_bass_kernel_spmd  # noqa: E402

B, S, H, I, E, TOPK, R = 4, 1024, 2048, 7168, 8, 2, 64
SCALING = 16.0 / 64.0
N_CORES = 8
N_TOK = B * S  # 4096
T = N_TOK // N_CORES  # 512 tokens per core
TC = T // 128  # 4 token chunks for transposed routing
HC = H // 128  # 16 contraction chunks over H
IT = I // 128  # 56 chunks over I
IT2 = IT // 2
HC2 = HC // 2
E2 = E // 2
TDPSUM = os.environ.get("TDPSUM", "1") == "1"

F32 = mybir.dt.float32
F32R = mybir.dt.float32r
BF16 = mybir.dt.bfloat16
BFNP = ml_dtypes.bfloat16
AF = mybir.ActivationFunctionType
OP = mybir.AluOpType
AX = mybir.AxisListType


def build_module() -> bass.Bass:
    nc = bacc.Bacc("TRN2", target_bir_lowering=False)

    # ---- dram I/O (per core) ----
    xT32 = nc.dram_tensor("xT32", [H, T], F32R, kind="ExternalInput")
    xTb = nc.dram_tensor("xTb", [H, T], BF16, kind="ExternalInput")
    conf_wt = nc.dram_tensor("conf_wt", [H, E], F32, kind="ExternalInput")
    conf_b = nc.dram_tensor("conf_b", [E, 1], F32, kind="ExternalInput")
    wealth = nc.dram_tensor("wealth", [E, 1], F32, kind="ExternalInput")
    # packed weights (see _host_prep for layouts); leading dim is the
    # chunk index so each load has a small partition stride
    guA = nc.dram_tensor("guA", [128, HC, E, 128], BF16, kind="ExternalInput")
    guBg = nc.dram_tensor("guBg", [IT2, 64, 2, E, 128], BF16, kind="ExternalInput")
    guBu = nc.dram_tensor("guBu", [IT2, 64, 2, E, 128], BF16, kind="ExternalInput")
    bgp = nc.dram_tensor("bgp", [IT2, 128, 2, HC, 128], BF16, kind="ExternalInput")
    bup = nc.dram_tensor("bup", [IT2, 128, 2, HC, 128], BF16, kind="ExternalInput")
    bdp = nc.dram_tensor("bdp", [HC2, 128, 2, IT, 128], BF16, kind="ExternalInput")
    dAp = nc.dram_tensor("dAp", [IT2, 128, 2, E, R], BF16, kind="ExternalInput")
    dBE = E2 if TDPSUM else E
    dBK = 128 if TDPSUM else 64
    dBp = nc.dram_tensor("dBp", [HC2, dBK, 2, dBE, 128], BF16, kind="ExternalInput")
    ident = nc.dram_tensor("ident", [128, 128], BF16, kind="ExternalInput")
    idf = nc.dram_tensor("idf", [128, 128], F32, kind="ExternalInput")
    bcast8 = nc.dram_tensor("bcast8", [E, E, 128], BF16, kind="ExternalInput")
    outT = nc.dram_tensor("outT", [H, T], F32, kind="ExternalOutput")

    with tile.TileContext(nc) as tc:
        with (
            tc.tile_pool(name="consts", bufs=1) as consts,
            tc.tile_pool(name="acc", bufs=IT) as accp,
            tc.tile_pool(name="xp", bufs=1) as xp,
            tc.tile_pool(name="tA", bufs=E) as tAp,
            tc.tile_pool(name="web", bufs=E) as webp,
            tc.tile_pool(name="tds", bufs=E) as tdsp,
        ):
            id_sb = consts.tile([128, 128], BF16)
            nc.sync.dma_start(out=id_sb, in_=ident[:, :])
            idf_sb = consts.tile([128, 128], F32)
            nc.sync.dma_start(out=idf_sb, in_=idf[:, :])
            bc8_sb = consts.tile([E, E, 128], BF16)
            nc.sync.dma_start(out=bc8_sb, in_=bcast8[:, :, :])
            cb_sb = consts.tile([E, 1], F32)
            nc.sync.dma_start(out=cb_sb, in_=conf_b[:, :])
            wl_sb = consts.tile([E, 1], F32)
            nc.sync.dma_start(out=wl_sb, in_=wealth[:, :])

            acc_t = [
                accp.tile([128, T], BF16, tag="acc", name=f"acc{i}")
                for i in range(IT)
            ]

            # ---------- load x (bf16 for compute) ----------
            xb_sb = xp.tile([128, HC, T], BF16)
            nc.sync.dma_start(
                out=xb_sb, in_=xTb[:, :].rearrange("(c p) t -> p c t", p=128)
            )

            we_b = []
            tAg = []
            tAu = []
            with (
                tc.tile_pool(name="rt", bufs=2) as rt,
                tc.tile_pool(name="xf", bufs=2) as xf,
                tc.tile_pool(name="wga", bufs=1) as wga,
                tc.tile_pool(name="prt", bufs=4, space="PSUM") as prt,
                tc.tile_pool(name="prs", bufs=2, space="PSUM") as prs,
            ):
                # ---------- tA = x @ [gate_A | up_A]: split 64-row tiles ----
                ga_sb = wga.tile([128, HC, E, 128], BF16, tag="guA")
                nc.sync.dma_start(out=ga_sb, in_=guA[:, :, :, :])
                for e in range(E):
                    p_tA = prt.tile([128, T], F32, tag="pbig")
                    for hc in range(HC):
                        nc.tensor.matmul(
                            p_tA,
                            ga_sb[:, hc, e, :],
                            xb_sb[:, hc, :],
                            start=(hc == 0),
                            stop=(hc == HC - 1),
                        )
                    tg = tAp.tile([64, T], BF16, tag="tAg", name=f"tAg{e}")
                    nc.scalar.copy(tg, p_tA[0:64, :])
                    tAg.append(tg)
                    tu = tAp.tile([64, T], BF16, tag="tAu", name=f"tAu{e}")
                    nc.scalar.copy(tu, p_tA[64:128, :])
                    tAu.append(tu)

                # ---------- confidence head (fp32, matches reference) ------
                cw_sb = rt.tile([128, HC, E], F32, tag="cw")
                nc.sync.dma_start(
                    out=cw_sb,
                    in_=conf_wt[:, :].rearrange("(c p) e -> p c e", p=128),
                )
                p_cf = prt.tile([128, T], F32, tag="pbig")
                for hc in range(HC):
                    xf_c = xf.tile([128, T], F32R, tag="xf")
                    nc.sync.dma_start(
                        out=xf_c, in_=xT32[hc * 128 : (hc + 1) * 128, :]
                    )
                    nc.tensor.matmul(
                        p_cf[0:E, :],
                        cw_sb[:, hc, :],
                        xf_c.bitcast(F32),
                        start=(hc == 0),
                        stop=(hc == HC - 1),
                    )
                conf = rt.tile([E, T], F32, tag="conf")
                nc.scalar.activation(conf, p_cf[0:E, :], AF.Sigmoid, bias=cb_sb)
                bids = rt.tile([E, T], F32, tag="bids")
                nc.vector.tensor_scalar(bids, conf, wl_sb, None, op0=OP.mult)

                # ---------- top-2 auction in token-major layout ----------
                # transpose bids into [128 tokens, E] chunks, reduce over the
                # free dim, build masks with per-partition scalars, transpose
                # the combine weights back.  No DRAM round trips.
                bT = rt.tile([128, TC, E], F32, tag="bT")
                m1 = rt.tile([128, TC], F32, tag="m1")
                m2 = rt.tile([128, TC], F32, tag="m2")
                msk1 = rt.tile([128, TC, E], F32, tag="msk1")
                msk2 = rt.tile([128, TC, E], F32, tag="msk2")
                b2 = rt.tile([128, TC, E], F32, tag="b2")
                weT = rt.tile([128, TC, E], F32, tag="weT")
                we8 = rt.tile([E, T], BF16, tag="we8")
                for c in range(TC):
                    p_bT = prs.tile([128, E], F32, tag="ptr")
                    nc.tensor.transpose(
                        p_bT, bids[:, c * 128 : (c + 1) * 128], idf_sb[0:E, 0:E]
                    )
                    nc.vector.tensor_copy(bT[:, c, :], p_bT)
                    nc.vector.tensor_reduce(
                        m1[:, c : c + 1], bT[:, c, :], AX.X, OP.max
                    )
                    nc.vector.tensor_scalar(
                        msk1[:, c, :], bT[:, c, :], m1[:, c : c + 1], None,
                        op0=OP.is_equal,
                    )
                    nc.vector.scalar_tensor_tensor(
                        b2[:, c, :], msk1[:, c, :], -1e6, bT[:, c, :],
                        op0=OP.mult, op1=OP.add,
                    )
                    nc.vector.tensor_reduce(
                        m2[:, c : c + 1], b2[:, c, :], AX.X, OP.max
                    )
                    nc.vector.tensor_scalar(
                        msk2[:, c, :], b2[:, c, :], m2[:, c : c + 1], None,
                        op0=OP.is_equal,
                    )
                d12 = rt.tile([128, TC], F32, tag="d12")
                nc.vector.tensor_sub(d12, m1, m2)
                w1 = rt.tile([128, TC], F32, tag="w1")
                nc.scalar.activation(w1, d12, AF.Sigmoid)
                w2 = rt.tile([128, TC], F32, tag="w2")
                nc.scalar.activation(w2, d12, AF.Sigmoid, scale=-1.0)
                for c in range(TC):
                    t1 = rt.tile([128, E], F32, tag="t1")
                    nc.vector.tensor_scalar(
                        t1, msk1[:, c, :], w1[:, c : c + 1], None, op0=OP.mult
                    )
                    t2 = rt.tile([128, E], F32, tag="t2")
                    nc.vector.tensor_scalar(
                        t2, msk2[:, c, :], w2[:, c : c + 1], None, op0=OP.mult
                    )
                    nc.vector.tensor_add(weT[:, c, :], t1, t2)
                    p_weT = prs.tile([E, 128], F32, tag="ptrb")
                    nc.tensor.transpose(p_weT, weT[:, c, :], idf_sb)
                    nc.vector.tensor_copy(
                        we8[:, c * 128 : (c + 1) * 128], p_weT
                    )
                # broadcast each expert's weight row to 128 partitions via
                # one-hot matmuls
                for e in range(E):
                    p_web = prt.tile([128, T], F32, tag="pbig")
                    nc.tensor.matmul(
                        p_web, bc8_sb[:, e, :], we8, start=True, stop=True
                    )
                    wt = webp.tile([128, T], BF16, tag="web", name=f"web{e}")
                    nc.scalar.copy(wt, p_web)
                    we_b.append(wt)

            # ---------- main loop over I chunk-pairs ----------
            with (
                tc.tile_pool(name="wgw", bufs=2) as wgw,
                tc.tile_pool(name="wb", bufs=2) as wbp,
                tc.tile_pool(name="wdA", bufs=3) as wdAp,
                tc.tile_pool(name="bsb", bufs=4) as bsb,
                tc.tile_pool(name="ew", bufs=4) as ew,
                tc.tile_pool(name="hwp", bufs=E + 1) as hwp,
                tc.tile_pool(name="ptd", bufs=E2, space="PSUM") as ptd,
                tc.tile_pool(name="pw", bufs=4, space="PSUM") as pw,
            ):
                if TDPSUM:
                    # open each down-LoRA PSUM bank once with a zeroing
                    # matmul (sets has_written across all 128 partitions);
                    # the per-expert dA matmuls then accumulate with
                    # start=False in their own partition halves.
                    zro = bsb.tile([128, 128], BF16, tag="zro")
                    nc.vector.memset(zro, 0)
                    p_td = [
                        ptd.tile([128, T], F32, tag="ptd", name=f"ptd{p}")
                        for p in range(E2)
                    ]
                    for p in range(E2):
                        nc.tensor.matmul(
                            p_td[p],
                            zro,
                            xb_sb[:, 0, :],
                            start=True,
                            stop=False,
                            skip_group_check=True,
                        )
                else:
                    td_bf = [
                        tdsp.tile([64, T], BF16, tag="tds", name=f"tds{q}")
                        for q in range(E)
                    ]
                for it2 in range(IT2):
                    bg_w = wgw.tile([128, 2, HC, 128], BF16, tag="bgw")
                    nc.sync.dma_start(out=bg_w, in_=bgp[it2, :, :, :, :])
                    bu_w = wgw.tile([128, 2, HC, 128], BF16, tag="buw")
                    nc.sync.dma_start(out=bu_w, in_=bup[it2, :, :, :, :])
                    gBg_s = wbp.tile([64, 2, E, 128], BF16, tag="gBg")
                    nc.sync.dma_start(out=gBg_s, in_=guBg[it2, :, :, :, :])
                    gBu_s = wbp.tile([64, 2, E, 128], BF16, tag="gBu")
                    nc.sync.dma_start(out=gBu_s, in_=guBu[it2, :, :, :, :])
                    dA_s = wdAp.tile([128, 2, E, R], BF16, tag="dA")
                    nc.sync.dma_start(out=dA_s, in_=dAp[it2, :, :, :, :])

                    for j in range(2):
                        it = 2 * it2 + j
                        p_bg = pw.tile([128, T], F32, tag="big")
                        p_bu = pw.tile([128, T], F32, tag="big")
                        for hc in range(HC):
                            nc.tensor.matmul(
                                p_bg,
                                bg_w[:, j, hc, :],
                                xb_sb[:, hc, :],
                                start=(hc == 0),
                                stop=(hc == HC - 1),
                            )
                        for hc in range(HC):
                            nc.tensor.matmul(
                                p_bu,
                                bu_w[:, j, hc, :],
                                xb_sb[:, hc, :],
                                start=(hc == 0),
                                stop=(hc == HC - 1),
                            )
                        bg_s = bsb.tile([128, T], BF16, tag="bgs")
                        nc.scalar.copy(bg_s, p_bg)
                        bu_s = bsb.tile([128, T], BF16, tag="bus")
                        nc.scalar.copy(bu_s, p_bu)

                        for e in range(E):
                            # g = base_g + lora_g in PSUM (identity trick;
                            # keeps the PSUM bank consumer on ACT so banks
                            # recycle fast -- a DVE-side add stalls the PE)
                            p_g = pw.tile([128, T], F32, tag="big")
                            nc.tensor.matmul(
                                p_g, id_sb, bg_s, start=True, stop=False
                            )
                            nc.tensor.matmul(
                                p_g,
                                gBg_s[:, j, e, :],
                                tAg[e],
                                start=False,
                                stop=True,
                            )
                            p_lu = pw.tile([128, T], F32, tag="big")
                            nc.tensor.matmul(
                                p_lu,
                                gBu_s[:, j, e, :],
                                tAu[e],
                                start=True,
                                stop=True,
                            )
                            sg = ew.tile([128, T], BF16, tag="sg")
                            nc.scalar.activation(sg, p_g, AF.Silu)
                            # u = lora_u + base_u; alternate the engine per
                            # expert parity so neither ACT nor DVE lags the
                            # PE inside the 8-expert stretch
                            u_t = ew.tile([128, T], BF16, tag="u")
                            if e % 2 == 0:
                                lu_s = ew.tile([128, T], BF16, tag="lu")
                                nc.scalar.copy(lu_s, p_lu)
                                nc.vector.tensor_add(u_t, lu_s, bu_s)
                            else:
                                nc.vector.scalar_tensor_tensor(
                                    u_t, p_lu, 1.0, bu_s,
                                    op0=OP.bypass, op1=OP.add,
                                )
                            h_t = ew.tile([128, T], BF16, tag="h")
                            nc.vector.tensor_mul(h_t, sg, u_t)
                            if e == 0:
                                hw_t = acc_t[it]
                                nc.vector.tensor_mul(hw_t, h_t, we_b[e])
                            else:
                                hw_t = hwp.tile([128, T], BF16, tag="hw")
                                nc.vector.tensor_mul(hw_t, h_t, we_b[e])
                                nc.vector.tensor_add(acc_t[it], acc_t[it], hw_t)
                            if TDPSUM:
                                nc.tensor.matmul(
                                    p_td[e // 2][
                                        (e % 2) * 64 : (e % 2) * 64 + 64, :
                                    ],
                                    dA_s[:, j, e, :],
                                    hw_t,
                                    start=False,
                                    stop=(it == IT - 1 and e == E - 1),
                                    skip_group_check=True,
                                )
                            else:
                                p_t1 = pw.tile([64, T], F32, tag="ptd1")
                                nc.tensor.matmul(
                                    p_t1, dA_s[:, j, e, :], hw_t,
                                    start=True, stop=True,
                                )
                                if it == 0:
                                    nc.vector.tensor_copy(td_bf[e], p_t1)
                                else:
                                    nc.vector.tensor_add(
                                        td_bf[e], td_bf[e], p_t1
                                    )

                # td: PSUM -> [128, T] bf16 expert-pair tiles (the dB
                # matmuls consume stacked pairs as K=128)
                if TDPSUM:
                    td_sb = []
                    for q in range(E2):
                        ts = tdsp.tile([128, T], BF16, tag="tds", name=f"tds{q}")
                        nc.scalar.copy(ts, p_td[q])
                        td_sb.append(ts)
                else:
                    td_sb = td_bf

            # ---------- down projection ----------
            with (
                tc.tile_pool(name="wd", bufs=2) as wd,
                tc.tile_pool(name="wdB", bufs=2) as wdB,
                tc.tile_pool(name="osb", bufs=3) as osb,
                tc.tile_pool(name="po", bufs=2, space="PSUM") as pop,
            ):
                for hc2 in range(HC2):
                    bd_s = wd.tile([128, 2, IT, 128], BF16, tag="bd")
                    nc.sync.dma_start(out=bd_s, in_=bdp[hc2, :, :, :, :])
                    dB_s = wdB.tile([dBK, 2, dBE, 128], BF16, tag="dB")
                    nc.sync.dma_start(out=dB_s, in_=dBp[hc2, :, :, :, :])
                    for j in range(2):
                        hc = 2 * hc2 + j
                        p_o = pop.tile([128, T], F32, tag="po")
                        for it in range(IT):
                            nc.tensor.matmul(
                                p_o,
                                bd_s[:, j, it, :],
                                acc_t[it],
                                start=(it == 0),
                                stop=False,
                            )
                        if TDPSUM:
                            for q in range(E2):
                                nc.tensor.matmul(
                                    p_o,
                                    dB_s[:, j, q, :],
                                    td_sb[q],
                                    start=False,
                                    stop=(q == E2 - 1),
                                )
                        else:
                            for e in range(E):
                                nc.tensor.matmul(
                                    p_o,
                                    dB_s[0:64, j, e, :],
                                    td_sb[e],
                                    start=False,
                                    stop=(e == E - 1),
                                )
                        o_s = osb.tile([128, T], F32, tag="o")
                        nc.scalar.copy(o_s, p_o)
                        nc.sync.dma_start(
                            out=outT[hc * 128 : (hc + 1) * 128, :], in_=o_s
                        )
    nc.compile()
    return nc


@functools.lru_cache(maxsize=1)
def _get_module():
    return build_module()


def _host_prep(inputs):
    f32 = np.float32
    x = np.ascontiguousarray(np.asarray(inputs["hidden_states"], f32)).reshape(
        N_TOK, H
    )
    gate_A = np.asarray(inputs["gate_A"], f32)
    gate_B = np.asarray(inputs["gate_B"], f32)
    up_A = np.asarray(inputs["up_A"], f32)
    up_B = np.asarray(inputs["up_B"], f32)
    down_A = np.asarray(inputs["down_A"], f32)
    down_B = np.asarray(inputs["down_B"], f32)
    bgate = np.asarray(inputs["base_gate"], f32)
    bup_w = np.asarray(inputs["base_up"], f32)
    bdown = np.asarray(inputs["base_down"], f32)

    # guA: [128, HC, E, 128]; [p, hc, e, r] = concat(A)[e, hc*128+p, r]
    guA_c = np.concatenate([gate_A, up_A], axis=2)  # [E, H, 2R]
    guAp = np.ascontiguousarray(
        guA_c.reshape(E, HC, 128, 2 * R).transpose(2, 1, 0, 3).astype(BFNP)
    )

    # guBg/guBu: [IT2, 64, 2, E, 128] (scaled)
    def pack_guB(w):  # [E, R, I]
        w = w * f32(SCALING)
        return np.ascontiguousarray(
            w.reshape(E, R, IT2, 2, 128).transpose(2, 1, 3, 0, 4).astype(BFNP)
        )

    guBgp = pack_guB(gate_B)
    guBup = pack_guB(up_B)

    # base gate/up: [IT2, 128, 2, HC, 128];
    # [it2, p, j, hc, i] = W[hc*128+p, (2*it2+j)*128+i]
    def pack_base(w):  # [H, I]
        return np.ascontiguousarray(
            w.reshape(HC, 128, IT2, 2, 128).transpose(2, 1, 3, 0, 4).astype(BFNP)
        )

    bgpk = pack_base(bgate)
    bupk = pack_base(bup_w)
    # base down: [HC2, 128, 2, IT, 128];
    # [hc2, p, j, it, h] = W[it*128+p, (2*hc2+j)*128+h]
    bdpk = np.ascontiguousarray(
        bdown.reshape(IT, 128, HC2, 2, 128).transpose(2, 1, 3, 0, 4).astype(BFNP)
    )
    # down_A: [IT2, 128, 2, E, R]
    dApk = np.ascontiguousarray(
        down_A.reshape(E, IT2, 2, 128, R).transpose(1, 3, 2, 0, 4).astype(BFNP)
    )
    if TDPSUM:
        # down_B: [HC2, 128, 2, E2, 128]; rows 0-63 expert 2q, 64-127
        # expert 2q+1 (stacked pairs contract as K=128)
        dBpk = np.ascontiguousarray(
            (down_B * f32(SCALING))
            .reshape(E2, 2, R, HC2, 2, 128)
            .transpose(3, 1, 2, 4, 0, 5)
            .reshape(HC2, 128, 2, E2, 128)
            .astype(BFNP)
        )
    else:
        # down_B: [HC2, 64, 2, E, 128] (scaled)
        dBpk = np.ascontiguousarray(
            (down_B * f32(SCALING))
            .reshape(E, R, HC2, 2, 128)
            .transpose(2, 1, 3, 0, 4)
            .astype(BFNP)
        )
    bc8 = np.zeros((E, E, 128), dtype=BFNP)
    for e in range(E):
        bc8[e, e, :] = BFNP(1.0)

    shared = {
        "conf_wt": np.ascontiguousarray(np.asarray(inputs["conf_W"], f32).T),
        "conf_b": np.ascontiguousarray(
            np.asarray(inputs["conf_b"], f32).reshape(E, 1)
        ),
        "wealth": np.ascontiguousarray(
            np.asarray(inputs["expert_wealth"], f32).reshape(E, 1)
        ),
        "guA": guAp,
        "guBg": guBgp,
        "guBu": guBup,
        "bgp": bgpk,
        "bup": bupk,
        "bdp": bdpk,
        "dAp": dApk,
        "dBp": dBpk,
        "ident": np.eye(128, dtype=BFNP),
        "idf": np.eye(128, dtype=f32),
        "bcast8": bc8,
    }
    in_maps = []
    for c in range(N_CORES):
        m = dict(shared)
        xc = np.ascontiguousarray(x[c * T : (c + 1) * T, :].T)
        m["xT32"] = xc
        m["xTb"] = np.ascontiguousarray(xc.astype(BFNP))
        in_maps.append(m)
    return in_maps


def kernel(**inputs) -> np.ndarray:
    nc = _get_module()
    in_maps = _host_prep(inputs)
    res = run_bass_kernel_spmd(nc, in_maps, core_ids=list(range(N_CORES)))
    parts = [np.asarray(r["outT"], np.float32).T for r in res.results]
    return np.concatenate(parts, axis=0).reshape(B, S, H)


# revision 25
# speedup vs baseline: 1.0158x; 1.0062x over previous
"""Trainium2 Bass kernel for nn_MixtureOfBidders.

Data-parallel over tokens (8 cores x 512 tokens), weights replicated.
On-device layout is transposed: [feature partitions, token free-dim].

v3 design (vs v1 baseline at 1.63 ms):
  - all heavy matmuls in bf16 (fp32r is ~1.5x slower on the PE and blocks
    fast weight load); conf/auction stays fp32 to match reference top-k
  - weights pre-packed on host chunk-major so every weight DMA is one
    fully-contiguous block per partition with a small (<=32 KiB)
    partition stride (large partition strides corrupt DMA on HW)
  - routing (top-2 auction) via PE transposes + free-dim reductions in
    token-major layout -- no DRAM round-trips
  - per-expert combine weights broadcast to 128 partitions with one-hot
    matmuls instead of DRAM-bounce DMAs
  - down-LoRA rank partials accumulate in PSUM across all I-chunks
    (2 experts col-packed per bank; opened once by a zeroing matmul)
  - g = base + lora computed via identity-matmul accumulation in PSUM
    (PE-side add); u-path add runs ACT copy + DVE bf16 add
"""

import functools
import os
import sys

import numpy as np

sys.path.insert(0, "/opt/trn_rl_repo")

import ml_dtypes  # noqa: E402

import concourse.bass as bass  # noqa: E402
from concourse import bacc  # noqa: E402
import concourse.mybir as mybir  # noqa: E402
import concourse.tile as tile  # noqa: E402
from concourse.bass_utils import run_bass_kernel_spmd  # noqa: E402

B, S, H, I, E, TOPK, R = 4, 1024, 2048, 7168, 8, 2, 64
SCALING = 16.0 / 64.0
N_CORES = 8
N_TOK = B * S  # 4096
T = N_TOK // N_CORES  # 512 tokens per core
TC = T // 128  # 4 token chunks for transposed routing
HC = H // 128  # 16 contraction chunks over H
IT = I // 128  # 56 chunks over I
IT2 = IT // 2
HC2 = HC // 2
E2 = E // 2
TDPSUM = os.environ.get("TDPSUM", "1") == "1"

F32 = mybir.dt.float32
F32R = mybir.dt.float32r
BF16 = mybir.dt.bfloat16
BFNP = ml_dtypes.bfloat16
AF = mybir.ActivationFunctionType
OP = mybir.AluOpType
AX = mybir.AxisListType


def build_module() -> bass.Bass:
    nc = bacc.Bacc("TRN2", target_bir_lowering=False)

    # ---- dram I/O (per core) ----
    xT32 = nc.dram_tensor("xT32", [H, T], F32R, kind="ExternalInput")
    xTb = nc.dram_tensor("xTb", [H, T], BF16, kind="ExternalInput")
    conf_wt = nc.dram_tensor("conf_wt", [H, E], F32, kind="ExternalInput")
    conf_b = nc.dram_tensor("conf_b", [E, 1], F32, kind="ExternalInput")
    wealth = nc.dram_tensor("wealth", [E, 1], F32, kind="ExternalInput")
    # packed weights (see _host_prep for layouts); leading dim is the
    # chunk index so each load has a small partition stride
    guA = nc.dram_tensor("guA", [128, HC, E, 128], BF16, kind="ExternalInput")
    guBg = nc.dram_tensor("guBg", [IT2, 64, 2, E, 128], BF16, kind="ExternalInput")
    guBu = nc.dram_tensor("guBu", [IT2, 64, 2, E, 128], BF16, kind="ExternalInput")
    bgp = nc.dram_tensor("bgp", [IT2, 128, 2, HC, 128], BF16, kind="ExternalInput")
    bup = nc.dram_tensor("bup", [IT2, 128, 2, HC, 128], BF16, kind="ExternalInput")
    bdp = nc.dram_tensor("bdp", [HC2, 128, 2, IT, 128], BF16, kind="ExternalInput")
    dAp = nc.dram_tensor("dAp", [IT2, 128, 2, E, R], BF16, kind="ExternalInput")
    dBE = E2 if TDPSUM else E
    dBK = 128 if TDPSUM else 64
    dBp = nc.dram_tensor("dBp", [HC2, dBK, 2, dBE, 128], BF16, kind="ExternalInput")
    ident = nc.dram_tensor("ident", [128, 128], BF16, kind="ExternalInput")
    idf = nc.dram_tensor("idf", [128, 128], F32, kind="ExternalInput")
    bcast8 = nc.dram_tensor("bcast8", [E, E, 128], BF16, kind="ExternalInput")
    outT = nc.dram_tensor("outT", [H, T], F32, kind="ExternalOutput")

    with tile.TileContext(nc) as tc:
        with (
            tc.tile_pool(name="consts", bufs=1) as consts,
            tc.tile_pool(name="acc", bufs=IT) as accp,
            tc.tile_pool(name="xp", bufs=1) as xp,
            tc.tile_pool(name="tA", bufs=E) as tAp,
            tc.tile_pool(name="web", bufs=E) as webp,
            tc.tile_pool(name="tds", bufs=E) as tdsp,
        ):
            id_sb = consts.tile([128, 128], BF16)
            nc.sync.dma_start(out=id_sb, in_=ident[:, :])
            idf_sb = consts.tile([128, 128], F32)
            nc.sync.dma_start(out=idf_sb, in_=idf[:, :])
            bc8_sb = consts.tile([E, E, 128], BF16)
            nc.sync.dma_start(out=bc8_sb, in_=bcast8[:, :, :])
            cb_sb = consts.tile([E, 1], F32)
            nc.sync.dma_start(out=cb_sb, in_=conf_b[:, :])
            wl_sb = consts.tile([E, 1], F32)
            nc.sync.dma_start(out=wl_sb, in_=wealth[:, :])

            acc_t = [
                accp.tile([128, T], BF16, tag="acc", name=f"acc{i}")
                for i in range(IT)
            ]

            # ---------- load x (bf16 for compute) ----------
            xb_sb = xp.tile([128, HC, T], BF16)
            nc.sync.dma_start(
                out=xb_sb, in_=xTb[:, :].rearrange("(c p) t -> p c t", p=128)
            )

            we_b = []
            tAg = []
            tAu = []
            with (
                tc.tile_pool(name="rt", bufs=2) as rt,
                tc.tile_pool(name="xf", bufs=2) as xf,
                tc.tile_pool(name="wga", bufs=1) as wga,
                tc.tile_pool(name="prt", bufs=4, space="PSUM") as prt,
                tc.tile_pool(name="prs", bufs=2, space="PSUM") as prs,
            ):
                # ---------- tA = x @ [gate_A | up_A]: split 64-row tiles ----
                ga_sb = wga.tile([128, HC, E, 128], BF16, tag="guA")
                nc.sync.dma_start(out=ga_sb, in_=guA[:, :, :, :])
                for e in range(E):
                    p_tA = prt.tile([128, T], F32, tag="pbig")
                    for hc in range(HC):
                        nc.tensor.matmul(
                            p_tA,
                            ga_sb[:, hc, e, :],
                            xb_sb[:, hc, :],
                            start=(hc == 0),
                            stop=(hc == HC - 1),
                        )
                    tg = tAp.tile([64, T], BF16, tag="tAg", name=f"tAg{e}")
                    nc.scalar.copy(tg, p_tA[0:64, :])
                    tAg.append(tg)
                    tu = tAp.tile([64, T], BF16, tag="tAu", name=f"tAu{e}")
                    nc.scalar.copy(tu, p_tA[64:128, :])
                    tAu.append(tu)

                # ---------- confidence head (fp32, matches reference) ------
                cw_sb = rt.tile([128, HC, E], F32, tag="cw")
                nc.sync.dma_start(
                    out=cw_sb,
                    in_=conf_wt[:, :].rearrange("(c p) e -> p c e", p=128),
                )
                p_cf = prt.tile([128, T], F32, tag="pbig")
                for hc in range(HC):
                    xf_c = xf.tile([128, T], F32R, tag="xf")
                    nc.sync.dma_start(
                        out=xf_c, in_=xT32[hc * 128 : (hc + 1) * 128, :]
                    )
                    nc.tensor.matmul(
                        p_cf[0:E, :],
                        cw_sb[:, hc, :],
                        xf_c.bitcast(F32),
                        start=(hc == 0),
                        stop=(hc == HC - 1),
                    )
                conf = rt.tile([E, T], F32, tag="conf")
                nc.scalar.activation(conf, p_cf[0:E, :], AF.Sigmoid, bias=cb_sb)
                bids = rt.tile([E, T], F32, tag="bids")
                nc.vector.tensor_scalar(bids, conf, wl_sb, None, op0=OP.mult)

                # ---------- top-2 auction in token-major layout ----------
                # transpose bids into [128 tokens, E] chunks, reduce over the
                # free dim, build masks with per-partition scalars, transpose
                # the combine weights back.  No DRAM round trips.
                bT = rt.tile([128, TC, E], F32, tag="bT")
                m1 = rt.tile([128, TC], F32, tag="m1")
                m2 = rt.tile([128, TC], F32, tag="m2")
                msk1 = rt.tile([128, TC, E], F32, tag="msk1")
                msk2 = rt.tile([128, TC, E], F32, tag="msk2")
                b2 = rt.tile([128, TC, E], F32, tag="b2")
                weT = rt.tile([128, TC, E], F32, tag="weT")
                we8 = rt.tile([E, T], BF16, tag="we8")
                for c in range(TC):
                    p_bT = prs.tile([128, E], F32, tag="ptr")
                    nc.tensor.transpose(
                        p_bT, bids[:, c * 128 : (c + 1) * 128], idf_sb[0:E, 0:E]
                    )
                    nc.vector.tensor_copy(bT[:, c, :], p_bT)
                    nc.vector.tensor_reduce(
                        m1[:, c : c + 1], bT[:, c, :], AX.X, OP.max
                    )
                    nc.vector.tensor_scalar(
                        msk1[:, c, :], bT[:, c, :], m1[:, c : c + 1], None,
                        op0=OP.is_equal,
                    )
                    nc.vector.scalar_tensor_tensor(
                        b2[:, c, :], msk1[:, c, :], -1e6, bT[:, c, :],
                        op0=OP.mult, op1=OP.add,
                    )
                    nc.vector.tensor_reduce(
                        m2[:, c : c + 1], b2[:, c, :], AX.X, OP.max
                    )
                    nc.vector.tensor_scalar(
                        msk2[:, c, :], b2[:, c, :], m2[:, c : c + 1], None,
                        op0=OP.is_equal,
                    )
                d12 = rt.tile([128, TC], F32, tag="d12")
                nc.vector.tensor_sub(d12, m1, m2)
                w1 = rt.tile([128, TC], F32, tag="w1")
                nc.scalar.activation(w1, d12, AF.Sigmoid)
                w2 = rt.tile([128, TC], F32, tag="w2")
                nc.scalar.activation(w2, d12, AF.Sigmoid, scale=-1.0)
                for c in range(TC):
                    t1 = rt.tile([128, E], F32, tag="t1")
                    nc.vector.tensor_scalar(
                        t1, msk1[:, c, :], w1[:, c : c + 1], None, op0=OP.mult
                    )
                    t2 = rt.tile([128, E], F32, tag="t2")
                    nc.vector.tensor_scalar(
                        t2, msk2[:, c, :], w2[:, c : c + 1], None, op0=OP.mult
                    )
                    nc.vector.tensor_add(weT[:, c, :], t1, t2)
                    p_weT = prs.tile([E, 128], F32, tag="ptrb")
                    nc.tensor.transpose(p_weT, weT[:, c, :], idf_sb)
                    nc.vector.tensor_copy(
                        we8[:, c * 128 : (c + 1) * 128], p_weT
                    )
                # broadcast each expert's weight row to 128 partitions via
                # one-hot matmuls
                for e in range(E):
                    p_web = prt.tile([128, T], F32, tag="pbig")
                    nc.tensor.matmul(
                        p_web, bc8_sb[:, e, :], we8, start=True, stop=True
                    )
                    wt = webp.tile([128, T], BF16, tag="web", name=f"web{e}")
                    nc.scalar.copy(wt, p_web)
                    we_b.append(wt)

            # ---------- main loop over I chunk-pairs ----------
            with (
                tc.tile_pool(name="wgw", bufs=2) as wgw,
                tc.tile_pool(name="wb", bufs=3) as wbp,
                tc.tile_pool(name="wdA", bufs=3) as wdAp,
                tc.tile_pool(name="bsb", bufs=4) as bsb,
                tc.tile_pool(name="ew", bufs=4) as ew,
                tc.tile_pool(name="hwp", bufs=E + 1) as hwp,
                tc.tile_pool(name="ptd", bufs=E2, space="PSUM") as ptd,
                tc.tile_pool(name="pw", bufs=4, space="PSUM") as pw,
            ):
                if TDPSUM:
                    # open each down-LoRA PSUM bank once with a zeroing
                    # matmul (sets has_written across all 128 partitions);
                    # the per-expert dA matmuls then accumulate with
                    # start=False in their own partition halves.
                    zro = bsb.tile([128, 128], BF16, tag="zro")
                    nc.vector.memset(zro, 0)
                    p_td = [
                        ptd.tile([128, T], F32, tag="ptd", name=f"ptd{p}")
                        for p in range(E2)
                    ]
                    for p in range(E2):
                        nc.tensor.matmul(
                            p_td[p],
                            zro,
                            xb_sb[:, 0, :],
                            start=True,
                            stop=False,
                            skip_group_check=True,
                        )
                else:
                    td_bf = [
                        tdsp.tile([64, T], BF16, tag="tds", name=f"tds{q}")
                        for q in range(E)
                    ]
                for it2 in range(IT2):
                    bg_w = wgw.tile([128, 2, HC, 128], BF16, tag="bgw")
                    nc.sync.dma_start(out=bg_w, in_=bgp[it2, :, :, :, :])
                    bu_w = wgw.tile([128, 2, HC, 128], BF16, tag="buw")
                    nc.sync.dma_start(out=bu_w, in_=bup[it2, :, :, :, :])
                    gBg_s = wbp.tile([64, 2, E, 128], BF16, tag="gBg")
                    nc.sync.dma_start(out=gBg_s, in_=guBg[it2, :, :, :, :])
                    gBu_s = wbp.tile([64, 2, E, 128], BF16, tag="gBu")
                    nc.sync.dma_start(out=gBu_s, in_=guBu[it2, :, :, :, :])
                    dA_s = wdAp.tile([128, 2, E, R], BF16, tag="dA")
                    nc.sync.dma_start(out=dA_s, in_=dAp[it2, :, :, :, :])

                    for j in range(2):
                        it = 2 * it2 + j
                        p_bg = pw.tile([128, T], F32, tag="big")
                        p_bu = pw.tile([128, T], F32, tag="big")
                        for hc in range(HC):
                            nc.tensor.matmul(
                                p_bg,
                                bg_w[:, j, hc, :],
                                xb_sb[:, hc, :],
                                start=(hc == 0),
                                stop=(hc == HC - 1),
                            )
                        for hc in range(HC):
                            nc.tensor.matmul(
                                p_bu,
                                bu_w[:, j, hc, :],
                                xb_sb[:, hc, :],
                                start=(hc == 0),
                                stop=(hc == HC - 1),
                            )
                        bg_s = bsb.tile([128, T], BF16, tag="bgs")
                        nc.scalar.copy(bg_s, p_bg)
                        bu_s = bsb.tile([128, T], BF16, tag="bus")
                        nc.scalar.copy(bu_s, p_bu)

                        for e in range(E):
                            # g = base_g + lora_g in PSUM (identity trick;
                            # keeps the PSUM bank consumer on ACT so banks
                            # recycle fast -- a DVE-side add stalls the PE)
                            p_g = pw.tile([128, T], F32, tag="big")
                            nc.tensor.matmul(
                                p_g, id_sb, bg_s, start=True, stop=False
                            )
                            nc.tensor.matmul(
                                p_g,
                                gBg_s[:, j, e, :],
                                tAg[e],
                                start=False,
                                stop=True,
                            )
                            p_lu = pw.tile([128, T], F32, tag="big")
                            nc.tensor.matmul(
                                p_lu,
                                gBu_s[:, j, e, :],
                                tAu[e],
                                start=True,
                                stop=True,
                            )
                            sg = ew.tile([128, T], BF16, tag="sg")
                            nc.scalar.activation(sg, p_g, AF.Silu)
                            # u = lora_u + base_u; alternate the engine per
                            # expert parity so neither ACT nor DVE lags the
                            # PE inside the 8-expert stretch
                            u_t = ew.tile([128, T], BF16, tag="u")
                            if e % 2 == 0:
                                lu_s = ew.tile([128, T], BF16, tag="lu")
                                nc.scalar.copy(lu_s, p_lu)
                                nc.vector.tensor_add(u_t, lu_s, bu_s)
                            else:
                                nc.vector.scalar_tensor_tensor(
                                    u_t, p_lu, 1.0, bu_s,
                                    op0=OP.bypass, op1=OP.add,
                                )
                            h_t = ew.tile([128, T], BF16, tag="h")
                            nc.vector.tensor_mul(h_t, sg, u_t)
                            if e == 0:
                                hw_t = acc_t[it]
                                nc.vector.tensor_mul(hw_t, h_t, we_b[e])
                            else:
                                hw_t = hwp.tile([128, T], BF16, tag="hw")
                                nc.vector.tensor_mul(hw_t, h_t, we_b[e])
                                nc.vector.tensor_add(acc_t[it], acc_t[it], hw_t)
                            if TDPSUM:
                                nc.tensor.matmul(
                                    p_td[e // 2][
                                        (e % 2) * 64 : (e % 2) * 64 + 64, :
                                    ],
                                    dA_s[:, j, e, :],
                                    hw_t,
                                    start=False,
                                    stop=(it == IT - 1 and e == E - 1),
                                    skip_group_check=True,
                                )
                            else:
                                p_t1 = pw.tile([64, T], F32, tag="ptd1")
                                nc.tensor.matmul(
                                    p_t1, dA_s[:, j, e, :], hw_t,
                                    start=True, stop=True,
                                )
                                if it == 0:
                                    nc.vector.tensor_copy(td_bf[e], p_t1)
                                else:
                                    nc.vector.tensor_add(
                                        td_bf[e], td_bf[e], p_t1
                                    )

                # td: PSUM -> [128, T] bf16 expert-pair tiles (the dB
                # matmuls consume stacked pairs as K=128)
                if TDPSUM:
                    td_sb = []
                    for q in range(E2):
                        ts = tdsp.tile([128, T], BF16, tag="tds", name=f"tds{q}")
                        nc.scalar.copy(ts, p_td[q])
                        td_sb.append(ts)
                else:
                    td_sb = td_bf

            # ---------- down projection ----------
            with (
                tc.tile_pool(name="wd", bufs=2) as wd,
                tc.tile_pool(name="wdB", bufs=2) as wdB,
                tc.tile_pool(name="osb", bufs=4) as osb,
                tc.tile_pool(name="po", bufs=3, space="PSUM") as pop,
            ):
                for hc2 in range(HC2):
                    bd_s = wd.tile([128, 2, IT, 128], BF16, tag="bd")
                    nc.sync.dma_start(out=bd_s, in_=bdp[hc2, :, :, :, :])
                    dB_s = wdB.tile([dBK, 2, dBE, 128], BF16, tag="dB")
                    nc.sync.dma_start(out=dB_s, in_=dBp[hc2, :, :, :, :])
                    for j in range(2):
                        hc = 2 * hc2 + j
                        p_o = pop.tile([128, T], F32, tag="po")
                        for it in range(IT):
                            nc.tensor.matmul(
                                p_o,
                                bd_s[:, j, it, :],
                                acc_t[it],
                                start=(it == 0),
                                stop=False,
                            )
                        if TDPSUM:
                            for q in range(E2):
                                nc.tensor.matmul(
                                    p_o,
                                    dB_s[:, j, q, :],
                                    td_sb[q],
                                    start=False,
                                    stop=(q == E2 - 1),
                                )
                        else:
                            for e in range(E):
                                nc.tensor.matmul(
                                    p_o,
                                    dB_s[0:64, j, e, :],
                                    td_sb[e],
                                    start=False,
                                    stop=(e == E - 1),
                                )
                        o_s = osb.tile([128, T], F32, tag="o")
                        nc.scalar.copy(o_s, p_o)
                        nc.sync.dma_start(
                            out=outT[hc * 128 : (hc + 1) * 128, :], in_=o_s
                        )
    nc.compile()
    return nc


@functools.lru_cache(maxsize=1)
def _get_module():
    return build_module()


def _host_prep(inputs):
    f32 = np.float32
    x = np.ascontiguousarray(np.asarray(inputs["hidden_states"], f32)).reshape(
        N_TOK, H
    )
    gate_A = np.asarray(inputs["gate_A"], f32)
    gate_B = np.asarray(inputs["gate_B"], f32)
    up_A = np.asarray(inputs["up_A"], f32)
    up_B = np.asarray(inputs["up_B"], f32)
    down_A = np.asarray(inputs["down_A"], f32)
    down_B = np.asarray(inputs["down_B"], f32)
    bgate = np.asarray(inputs["base_gate"], f32)
    bup_w = np.asarray(inputs["base_up"], f32)
    bdown = np.asarray(inputs["base_down"], f32)

    # guA: [128, HC, E, 128]; [p, hc, e, r] = concat(A)[e, hc*128+p, r]
    guA_c = np.concatenate([gate_A, up_A], axis=2)  # [E, H, 2R]
    guAp = np.ascontiguousarray(
        guA_c.reshape(E, HC, 128, 2 * R).transpose(2, 1, 0, 3).astype(BFNP)
    )

    # guBg/guBu: [IT2, 64, 2, E, 128] (scaled)
    def pack_guB(w):  # [E, R, I]
        w = w * f32(SCALING)
        return np.ascontiguousarray(
            w.reshape(E, R, IT2, 2, 128).transpose(2, 1, 3, 0, 4).astype(BFNP)
        )

    guBgp = pack_guB(gate_B)
    guBup = pack_guB(up_B)

    # base gate/up: [IT2, 128, 2, HC, 128];
    # [it2, p, j, hc, i] = W[hc*128+p, (2*it2+j)*128+i]
    def pack_base(w):  # [H, I]
        return np.ascontiguousarray(
            w.reshape(HC, 128, IT2, 2, 128).transpose(2, 1, 3, 0, 4).astype(BFNP)
        )

    bgpk = pack_base(bgate)
    bupk = pack_base(bup_w)
    # base down: [HC2, 128, 2, IT, 128];
    # [hc2, p, j, it, h] = W[it*128+p, (2*hc2+j)*128+h]
    bdpk = np.ascontiguousarray(
        bdown.reshape(IT, 128, HC2, 2, 128).transpose(2, 1, 3, 0, 4).astype(BFNP)
    )
    # down_A: [IT2, 128, 2, E, R]
    dApk = np.ascontiguousarray(
        down_A.reshape(E, IT2, 2, 128, R).transpose(1, 3, 2, 0, 4).astype(BFNP)
    )
    if TDPSUM:
        # down_B: [HC2, 128, 2, E2, 128]; rows 0-63 expert 2q, 64-127
        # expert 2q+1 (stacked pairs contract as K=128)
        dBpk = np.ascontiguousarray(
            (down_B * f32(SCALING))
            .reshape(E2, 2, R, HC2, 2, 128)
            .transpose(3, 1, 2, 4, 0, 5)
            .reshape(HC2, 128, 2, E2, 128)
            .astype(BFNP)
        )
    else:
        # down_B: [HC2, 64, 2, E, 128] (scaled)
        dBpk = np.ascontiguousarray(
            (down_B * f32(SCALING))
            .reshape(E, R, HC2, 2, 128)
            .transpose(2, 1, 3, 0, 4)
            .astype(BFNP)
        )
    bc8 = np.zeros((E, E, 128), dtype=BFNP)
    for e in range(E):
        bc8[e, e, :] = BFNP(1.0)

    shared = {
        "conf_wt": np.ascontiguousarray(np.asarray(inputs["conf_W"], f32).T),
        "conf_b": np.ascontiguousarray(
            np.asarray(inputs["conf_b"], f32).reshape(E, 1)
        ),
        "wealth": np.ascontiguousarray(
            np.asarray(inputs["expert_wealth"], f32).reshape(E, 1)
        ),
        "guA": guAp,
        "guBg": guBgp,
        "guBu": guBup,
        "bgp": bgpk,
        "bup": bupk,
        "bdp": bdpk,
        "dAp": dApk,
        "dBp": dBpk,
        "ident": np.eye(128, dtype=BFNP),
        "idf": np.eye(128, dtype=f32),
        "bcast8": bc8,
    }
    in_maps = []
    for c in range(N_CORES):
        m = dict(shared)
        xc = np.ascontiguousarray(x[c * T : (c + 1) * T, :].T)
        m["xT32"] = xc
        m["xTb"] = np.ascontiguousarray(xc.astype(BFNP))
        in_maps.append(m)
    return in_maps


def kernel(**inputs) -> np.ndarray:
    nc = _get_module()
    in_maps = _host_prep(inputs)
    res = run_bass_kernel_spmd(nc, in_maps, core_ids=list(range(N_CORES)))
    parts = [np.asarray(r["outT"], np.float32).T for r in res.results]
    return np.concatenate(parts, axis=0).reshape(B, S, H)
